# revision 14
# baseline (speedup 1.0000x reference)
"""Trainium2 Bass kernel for nn_AndLayer (permutation-based AND layer).

Math (see reference):
    tk = tanh(kernel)                 # [448, C=128]
    q  = 1 - tk^2
    For each batch b and permutation k=(o0,o1) of 8 objects (K=56 perms):
        in_vec[448] = [nullary(64) | unary[o0](128) | unary[o1](128)
                       | binary[o0,o1'](64) | binary[o1,o0'](64)]
        conj[b,k,c] = min_i (in_vec[i]*tk[i,c] + q[i,c])
        out[b,c]    = max_k conj[b,k,c]

Decomposition (exact):
    nmin[b,c]     = min over nullary 64 rows        (shared by all k)
    umin[b,o,pos] = min over unary   128 rows       (16 combos per b)
    bmin[b,k,c]   = min over binary 128 rows        (per k)
    conj = min(bmin[k], umin0[o0], umin1[o1], nmin); out = max_k conj

Device strategy (per core, data-parallel over B: 4 batches/core):
    One matmul per 64-pred half-tile computes in*tk + q directly: the
    stationary stacks [tk_half ; 1-tk_half^2] (128 rows) and the moving
    operand stacks [diag(in_half) ; I64].  PSUM lands transposed
    ([c, tile*pred]) so the min-reduce is a free-axis reduce.

    v4 drain design (engine-rate driven):
    - W-waves: Scalar copies the 2048-elem PSUM wave to SBUF bf16
      (1.9us, ACT is the cheapest PSUM toucher), Vector folds with
      4x-mode scalar_tensor_tensor (bf16/SBUF/packed => 0.26ns/elem),
      final level = per-run tensor_reduce writing straight into the
      grid with a strided 4D AP (TR is the only DVE op allowing >2
      free output dims).
    - Z-waves: one Vector tensor_reduce (XY) directly from PSUM
      (2.3us) -- used where Scalar is busy (prep, start).
    - Binary waves are grouped by pair-diagonal (j-i = const): the
      per-tile mins of a run land on an affine AP in the [b,i,j] grid
      (offset 9i+g, strides i:9, b:64, d:7g), so no scatter pass at
      all (v3 spent ~5us of GpSimd+Vector on scatter copies).
    - Batches are processed in two halves (rounds); each half's
      combine + output DMA overlaps the other half's waves, removing
      the v3 serial tail.
"""

import itertools
import os
import sys

import numpy as np

for _p in ("/opt/trn_rl_repo", "/root/.axon_site/_ro/trn_rl_repo"):
    if os.path.isdir(_p) and _p not in sys.path:
        sys.path.insert(0, _p)

import concourse.bass as bass  # noqa: E402
import concourse.bacc as bacc  # noqa: E402
import concourse.mybir as mybir  # noqa: E402
import concourse.tile as tile  # noqa: E402
from concourse.bass import AP  # noqa: E402
from concourse.bass_utils import run_bass_kernel_spmd  # noqa: E402

import ml_dtypes  # noqa: E402

BF16 = ml_dtypes.bfloat16

# Problem constants (hardcoded per spec)
B, N, V = 32, 8, 2
P0, P1, P2, C = 64, 128, 64, 128
K = 56  # permutations of 2 from 8
NCORES = 8
BL = B // NCORES  # 4 batches per core
NBT = BL * K  # binary tiles per core = 224

F32 = mybir.dt.float32
BF16_T = mybir.dt.bfloat16
MIN_OP = mybir.AluOpType.min
MAX_OP = mybir.AluOpType.max
MULT_OP = mybir.AluOpType.mult

# chunk order in the rearranged kernel tensor (64-row chunks of the 448):
# unary (1..4), binary (5,6), nullary (0) last.  CHUNK[ci] = orig chunk.
CHUNK = [1, 2, 3, 4, 5, 6, 0]

# Binary pair sequence by diagonal: all pairs (i, i+g) sorted by (g, i).
P_SEQ = [(i, i + g) for g in range(1, 8) for i in range(0, 8 - g)]  # 28
NWAVE = 7  # binary waves per round, 4 pairs each
WAVE_PAIRS = [P_SEQ[4 * w : 4 * w + 4] for w in range(NWAVE)]


def _pidx(x, y):
    """binf flat index of perm (x, y): binary[b, x, adj(y)]."""
    return x * 7 + (y - (y > x))


def _wave_runs(pairs):
    """Split a wave's pair list into runs of (same gap, consecutive i).
    Returns [(g, i0, n, p0)]: gap, start i, run length, index in wave."""
    runs = []
    for idx, (i, j) in enumerate(pairs):
        g = j - i
        if runs and runs[-1][0] == g and runs[-1][1] + runs[-1][2] == i:
            runs[-1][2] += 1
        else:
            runs.append([g, i, 1, idx])
    return [tuple(r) for r in runs]


WAVE_RUNS = [_wave_runs(p) for p in WAVE_PAIRS]


def build_graph():
    nc = bacc.Bacc("TRN2", debug=False)

    kern_d = nc.declare_dram_parameter("kern", [128, 7 * 128], F32, isOutput=False)
    # aun: [h0 atlas 2048 | h1 atlas 2048 | anul 256]
    aun_d = nc.declare_dram_parameter("aun", [128, 4352], BF16_T, isOutput=False)
    # abin: (half, wave, pair4, b2, d2) blocks of 64
    abin_d = nc.declare_dram_parameter("abin", [128, NBT * 64], BF16_T, isOutput=False)
    out_d = nc.declare_dram_parameter("out", [128, BL], F32, isOutput=True)

    with tile.TileContext(nc) as tc:
        with (
            tc.tile_pool(name="const", bufs=1) as const,
            tc.tile_pool(name="drain", bufs=3) as dr,
            tc.tile_pool(name="fold", bufs=2) as fp,
            tc.tile_pool(name="psum", bufs=2, space="PSUM") as psum_pool,
        ):
            # tanh activation-table preload (overlaps input DMAs)
            dum = const.tile([128, 512], BF16_T, tag="dum")
            dout = const.tile([128, 8], BF16_T, tag="dout")
            nc.gpsimd.memset(dum[:], 0.0)
            nc.scalar.activation(
                dout[:], dum[:, 0:8], mybir.ActivationFunctionType.Tanh
            )
            # ---- input DMAs in need-order on the Sync queue ----
            raw = const.tile([128, 896], F32, tag="raw")
            aun_s = const.tile([128, 4352], BF16_T, tag="aun")
            abin_s = const.tile([128, NBT * 64], BF16_T, tag="abin")
            nc.sync.dma_start(raw[:, 0:256], kern_d[:, 0:256])  # u0 chunks
            nc.sync.dma_start(raw[:, 256:512], kern_d[:, 256:512])
            nc.sync.dma_start(aun_s[:, 0:2048], aun_d[:, 0:2048])  # h0 atlas
            nc.sync.dma_start(raw[:, 512:896], kern_d[:, 512:896])  # bin+nul
            nc.sync.dma_start(aun_s[:, 4096:4352], aun_d[:, 4096:4352])  # anul
            for lo, hi in ((0, 2048), (2048, 4096), (4096, 7168)):  # R1 atlas
                nc.sync.dma_start(abin_s[:, lo:hi], abin_d[:, lo:hi])
            nc.sync.dma_start(aun_s[:, 2048:4096], aun_d[:, 2048:4096])  # h1
            for lo, hi in ((7168, 9216), (9216, 11264), (11264, 14336)):
                nc.sync.dma_start(abin_s[:, lo:hi], abin_d[:, lo:hi])

            # ---- PE warmup: ~12 dummy matmuls span the DMA/prep lead-in,
            # so HAM is at K=8/8 when the first real waves arrive and the
            # early wave cascade fills at 216ns/mm instead of 427. ----
            for _ in range(12):
                wps = psum_pool.tile([128, 512], F32, tag="ps")
                nc.tensor.matmul(
                    wps[:], dum[:, 0:128], dum[:], start=True, stop=True
                )

            # ---- stationaries: st chunk ci = [tanh(rows); 1-tanh^2] ----
            st = const.tile([128, 896], BF16_T, tag="st")
            sq = const.tile([64, 896], F32, tag="sq")

            def prep(lo, hi):
                # tanh on all 128 partitions (top=bottom rows), square on
                # scalar, affine (1 - t^2) on gpsimd.
                nc.scalar.activation(
                    st[:, lo:hi], raw[:, lo:hi], mybir.ActivationFunctionType.Tanh
                )
                nc.scalar.activation(
                    sq[:, lo:hi], st[64:128, lo:hi],
                    mybir.ActivationFunctionType.Square,
                )
                nc.gpsimd.tensor_scalar(
                    st[64:128, lo:hi], sq[:, lo:hi], -1.0, 1.0,
                    MULT_OP, mybir.AluOpType.add,
                )

            stc = [st[:, ci * 128 : (ci + 1) * 128] for ci in range(7)]
            st_u0a, st_u0b, st_u1a, st_u1b, st_ba, st_bb, st_n = stc

            # ---- persistent accumulators ----
            um = const.tile([128, 64], BF16_T, tag="um")  # (b4, pos2, o8)
            nm = const.tile([128, BL], BF16_T, tag="nm")
            grid = [const.tile([128, 128], BF16_T, name=f"grid{h}", tag=f"grid{h}") for h in range(2)]
            umask = [const.tile([128, 128], BF16_T, name=f"umask{h}", tag=f"umask{h}") for h in range(2)]
            outf = const.tile([128, BL], F32, tag="outf")
            um4 = um[:].rearrange("p (b q o) -> p b q o", b=BL, q=2)

            for h in range(2):  # grid diagonal = -inf
                nc.vector.memset(
                    grid[h][:].rearrange("p (b c) -> p b c", b=2)[:, :, 0:64:9],
                    -3.0e38,
                )

            # ---------------- wave matmul emitters ----------------
            def binary_mms(h, w):
                """4 pairs x 2 b x 2 d tiles; psum = [A x16 | B x16]."""
                ps = psum_pool.tile([128, 2048], F32, tag="ps")
                base = h * 7168 + w * 1024
                for half in range(2):
                    nc.tensor.matmul(
                        ps[:, 512 * half : 512 * half + 512], st_ba,
                        abin_s[:, base + 512 * half : base + 512 * half + 512],
                        start=True, stop=True,
                    )
                for half in range(2):  # B: dir-flipped read (d: -64)
                    b_ap = AP(
                        tensor=abin_s[:].tensor,
                        offset=abin_s[:].offset + base + 512 * half + 64,
                        ap=[abin_s[:].ap[0], [128, 4], [-64, 2], [1, 64]],
                    )
                    nc.tensor.matmul(
                        ps[:, 1024 + 512 * half : 1536 + 512 * half], st_bb, b_ap,
                        start=True, stop=True,
                    )
                return ps

            def unary_mms(h, pos):
                """16 (b2, o8) tiles for half h, position pos."""
                ps = psum_pool.tile([128, 2048], F32, tag="ps")
                sa, sb = stc[2 * pos], stc[2 * pos + 1]
                base = 2048 * h
                for half in range(2):
                    nc.tensor.matmul(
                        ps[:, 512 * half : 512 * half + 512], sa,
                        aun_s[:, base + 512 * half : base + 512 * half + 512],
                        start=True, stop=True,
                    )
                for half in range(2):
                    nc.tensor.matmul(
                        ps[:, 1024 + 512 * half : 1536 + 512 * half], sb,
                        aun_s[:, base + 1024 + 512 * half : base + 1536 + 512 * half],
                        start=True, stop=True,
                    )
                return ps

            # ---------------- drain helpers ----------------
            def grid_out_ap(h, g, i0, n):
                """Strided AP into grid[h]: dims [i(n), lb(2), d(2)],
                strides [9, 64, 7g], offset 9*i0 + g."""
                gt = grid[h][:]
                return AP(
                    tensor=gt.tensor,
                    offset=gt.offset + 9 * i0 + g,
                    ap=[gt.ap[0], [9, n], [64, 2], [7 * g, 2]],
                )

            def bin_dsts(h, w, src_ap, tile_stride, red_axis):
                """Per-run reduces from src (tile-major) into grid[h]."""
                for g, i0, n, p0 in WAVE_RUNS[w]:
                    t0 = p0 * 4  # tiles per pair = b2*d2
                    sub = AP(
                        tensor=src_ap.tensor,
                        offset=src_ap.offset + t0 * tile_stride,
                        ap=[src_ap.ap[0], [tile_stride, 4 * n]] + src_ap.ap[2:],
                    )
                    nc.vector.tensor_reduce(
                        grid_out_ap(h, g, i0, n), sub, red_axis, MIN_OP
                    )

            def W_bin_ps(h, w):
                """PSUM phase: matmuls + scalar copy.  Returns the deferred
                SBUF fold closure (emitted one wave later so Z-wave TRs are
                not stuck behind folds in the Vector FIFO)."""
                ps = binary_mms(h, w)
                scr = dr.tile([128, 2048], BF16_T, tag="scr")
                nc.scalar.activation(
                    scr[:], ps[:], mybir.ActivationFunctionType.Copy
                )

                def folds():
                    f1 = fp.tile([128, 1024], BF16_T, tag="f1")
                    nc.vector.tensor_tensor(
                        f1[:], scr[:, 0:1024], scr[:, 1024:2048], MIN_OP
                    )
                    f1v = f1[:].rearrange("p (t c f) -> p t c f", c=2, f=32)
                    f2 = fp.tile([128, 512], BF16_T, tag="f2")
                    f2o = f2[:].rearrange("p (t f) -> p t f", f=32)
                    nc.vector.tensor_tensor(
                        f2o, f1v[:, :, 0], f1v[:, :, 1], MIN_OP
                    )
                    src = AP(
                        tensor=f2[:].tensor, offset=f2[:].offset,
                        ap=[f2[:].ap[0], [32, 16], [1, 32]],
                    )
                    bin_dsts(h, w, src, 32, mybir.AxisListType.X)

                return folds

            def Z_bin_mini(h, w, part):
                """Half of wave w (2 pairs = 8 tiles, 1024 psum), Z-drained."""
                ps = psum_pool.tile([128, 1024], F32, tag="ps")
                base = h * 7168 + w * 1024 + part * 512
                nc.tensor.matmul(
                    ps[:, 0:512], st_ba, abin_s[:, base : base + 512],
                    start=True, stop=True,
                )
                b_ap = AP(
                    tensor=abin_s[:].tensor,
                    offset=abin_s[:].offset + base + 64,
                    ap=[abin_s[:].ap[0], [128, 4], [-64, 2], [1, 64]],
                )
                nc.tensor.matmul(
                    ps[:, 512:1024], st_bb, b_ap, start=True, stop=True
                )
                src = AP(
                    tensor=ps[:].tensor, offset=ps[:].offset,
                    ap=[ps[:].ap[0], [64, 8], [512, 2], [1, 64]],
                )
                for g, i0, n, p0 in WAVE_RUNS[w]:
                    lo, hi_ = p0 * 2, (p0 + n) * 2  # d-pairs per pair
                    # keep only runs inside this half (pairs 2*part..2*part+1)
                    pl, ph = 2 * part, 2 * part + 2
                    rl, rh = max(p0, pl), min(p0 + n, ph)
                    if rl >= rh:
                        continue
                    sub = AP(
                        tensor=src.tensor,
                        offset=src.offset + (rl - pl) * 4 * 64,
                        ap=[src.ap[0], [64, 4 * (rh - rl)], [512, 2], [1, 64]],
                    )
                    nc.vector.tensor_reduce(
                        grid_out_ap(h, g, i0 + (rl - p0), rh - rl),
                        sub, mybir.AxisListType.XY, MIN_OP,
                    )

            def Z_bin(h, w):
                ps = binary_mms(h, w)
                # per-run: TR XY over [t, c2, f64] from PSUM -> grid
                src = AP(
                    tensor=ps[:].tensor, offset=ps[:].offset,
                    ap=[ps[:].ap[0], [64, 16], [1024, 2], [1, 64]],
                )
                bin_dsts(h, w, src, 64, mybir.AxisListType.XY)

            def um_dst(h, pos):
                return um4[:, 2 * h : 2 * h + 2, pos, :]

            def W_un_ps(h, pos):
                ps = unary_mms(h, pos)
                scr = dr.tile([128, 2048], BF16_T, tag="scr")
                nc.scalar.activation(
                    scr[:], ps[:], mybir.ActivationFunctionType.Copy
                )

                def folds():
                    f1 = fp.tile([128, 1024], BF16_T, tag="f1")
                    nc.vector.tensor_tensor(
                        f1[:], scr[:, 0:1024], scr[:, 1024:2048], MIN_OP
                    )
                    f1v = f1[:].rearrange("p (t c f) -> p t c f", c=2, f=32)
                    f2 = fp.tile([128, 512], BF16_T, tag="f2")
                    f2o = f2[:].rearrange("p (t f) -> p t f", f=32)
                    nc.vector.tensor_tensor(
                        f2o, f1v[:, :, 0], f1v[:, :, 1], MIN_OP
                    )
                    nc.vector.tensor_reduce(
                        um_dst(h, pos),
                        f2[:].rearrange("p (t f) -> p t f", f=32),
                        mybir.AxisListType.X, MIN_OP,
                    )

                return folds

            def Z_un_mini(h, pos, part):
                """One batch's unary tiles (8 tiles, 1024 psum)."""
                ps = psum_pool.tile([128, 1024], F32, tag="ps")
                sa, sb = stc[2 * pos], stc[2 * pos + 1]
                base = 2048 * h + 512 * part
                nc.tensor.matmul(
                    ps[:, 0:512], sa, aun_s[:, base : base + 512],
                    start=True, stop=True,
                )
                nc.tensor.matmul(
                    ps[:, 512:1024], sb,
                    aun_s[:, base + 1024 : base + 1536],
                    start=True, stop=True,
                )
                nc.vector.tensor_reduce(
                    um4[:, 2 * h + part, pos, :],
                    ps[:].rearrange("p (c t f) -> p t c f", c=2, f=64),
                    mybir.AxisListType.XY, MIN_OP,
                )

            def Z_un(h, pos):
                ps = unary_mms(h, pos)
                nc.vector.tensor_reduce(
                    um_dst(h, pos),
                    ps[:].rearrange("p (c t f) -> p t c f", c=2, f=64),
                    mybir.AxisListType.XY, MIN_OP,
                )

            def nullary_wave():
                ps = psum_pool.tile([128, 256], F32, tag="ps")
                nc.tensor.matmul(
                    ps[:], st_n, aun_s[:, 4096:4352], start=True, stop=True
                )
                nc.vector.tensor_reduce(
                    nm[:],
                    ps[:].rearrange("p (t f) -> p t f", f=64),
                    mybir.AxisListType.X, MIN_OP,
                )

            def build_umask(h):
                uh = umask[h][:].rearrange("p (b i j) -> p b i j", b=2, i=8)
                u0 = um4[:, 2 * h : 2 * h + 2, 0, :]
                u1 = um4[:, 2 * h : 2 * h + 2, 1, :]
                nc.vector.tensor_tensor(
                    uh,
                    u0.unsqueeze(3).to_broadcast((128, 2, 8, 8)),
                    u1.unsqueeze(2).to_broadcast((128, 2, 8, 8)),
                    MIN_OP,
                )
                nc.vector.tensor_tensor(
                    uh, uh,
                    nm[:, 2 * h : 2 * h + 2]
                    .unsqueeze(2).unsqueeze(3).to_broadcast((128, 2, 8, 8)),
                    MIN_OP,
                )

            def combine(h, dma_eng):
                gc = fp.tile([128, 128], BF16_T, tag="gc")
                nc.vector.tensor_tensor(gc[:], grid[h][:], umask[h][:], MIN_OP)
                nc.vector.tensor_reduce(
                    outf[:, 2 * h : 2 * h + 2],
                    gc[:].rearrange("p (b f) -> p b f", b=2),
                    mybir.AxisListType.X, MAX_OP,
                )
                dma_eng.dma_start(
                    out_d[:, 2 * h : 2 * h + 2], outf[:, 2 * h : 2 * h + 2]
                )

            # ---------------- schedule ----------------
            # Software-pipelined: each wave's PSUM phase (matmuls + scalar
            # copy or Z-TR) is emitted before the PREVIOUS wave's SBUF
            # folds, so Vector always frees PSUM slots before chewing on
            # fold work.  Z-waves sit where Scalar is busy (prep / start).
            prep(0, 256)       # u0a, u0b
            Z_un_mini(0, 0, 0)
            Z_un_mini(0, 0, 1)
            prep(256, 512)     # u1a, u1b
            fu0 = W_un_ps(0, 1)
            prep(512, 768)     # ba, bb
            prep(768, 896)     # n
            nullary_wave()
            Z_bin_mini(0, 0, 0)
            Z_bin_mini(0, 0, 1)
            fu0()
            build_umask(0)
            f1_ = W_bin_ps(0, 1)
            Z_bin_mini(0, 2, 0)
            Z_bin_mini(0, 2, 1)
            f1_()
            f3_ = W_bin_ps(0, 3)
            f4_ = W_bin_ps(0, 4)
            f3_()
            f5_ = W_bin_ps(0, 5)
            f4_()
            f6_ = W_bin_ps(0, 6)
            f5_()
            # Round 1 (batches 2-3) starts while round 0 drains
            Z_un_mini(1, 0, 0)
            Z_un_mini(1, 0, 1)
            f6_()
            combine(0, nc.sync)
            fu1 = W_un_ps(1, 1)
            g0 = W_bin_ps(1, 0)
            fu1()
            build_umask(1)
            g1 = W_bin_ps(1, 1)
            g0()
            Z_bin_mini(1, 2, 0)
            Z_bin_mini(1, 2, 1)
            g1()
            g3 = W_bin_ps(1, 3)
            g4 = W_bin_ps(1, 4)
            g3()
            g5 = W_bin_ps(1, 5)
            g4()
            Z_bin_mini(1, 6, 0)
            g5()
            Z_bin_mini(1, 6, 1)
            combine(1, nc.scalar)

    nc.compile()
    return nc


def _diag_blocks(scales):
    """scales [T, 64] -> [128, T*64] bf16: block t = [diag(scales[t]); I64]."""
    T = scales.shape[0]
    atlas = np.zeros((128, T * 64), dtype=BF16)
    t = np.arange(T)
    j = np.arange(64)
    cols = (t * 64)[:, None] + j[None, :]
    atlas[j[None, :], cols] = scales.astype(BF16)
    atlas[64 + j[None, :], cols] = 1.0
    return atlas


def make_core_inputs(nul, una, binf, ker):
    """Per-core in_map. nul [4,64], una [4,8,128], binf [4,56,64] f32."""
    # kern: [128, 896] f32, chunk order CHUNK, rows replicated in both halves
    kern = np.empty((128, 896), dtype=np.float32)
    for ci, ch in enumerate(CHUNK):
        rows = ker[64 * ch : 64 * ch + 64]  # [64, 128]
        kern[0:64, ci * 128 : (ci + 1) * 128] = rows
        kern[64:128, ci * 128 : (ci + 1) * 128] = rows
    # binary blocks in (half, wave, pair4, b2, d2) order
    sc = np.empty((2, NWAVE, 4, 2, 2, 64), dtype=np.float32)
    for h in range(2):
        for w in range(NWAVE):
            for pi, (i, j) in enumerate(WAVE_PAIRS[w]):
                for lb in range(2):
                    b = 2 * h + lb
                    sc[h, w, pi, lb, 0] = binf[b, _pidx(i, j)]
                    sc[h, w, pi, lb, 1] = binf[b, _pidx(j, i)]
    abin = _diag_blocks(sc.reshape(NBT, 64))
    # unary atlas per half: [A-blocks x16 | B-blocks x16]
    su = una.reshape(2, 16, 128)  # [h, (b2 o8), 128]
    su2 = np.concatenate([su[:, :, :64], su[:, :, 64:]], axis=1)  # [h, 32, 64]
    aun_u = _diag_blocks(su2.reshape(64, 64))
    anul = _diag_blocks(nul)  # [128, 4*64]
    aun = np.concatenate([aun_u, anul], axis=1)
    return {
        "kern": kern,
        "aun": np.ascontiguousarray(aun),
        "abin": np.ascontiguousarray(abin),
    }


LAST_RESULTS = None
_GRAPH_CACHE = {}


def get_graph():
    if "nc" not in _GRAPH_CACHE:
        _GRAPH_CACHE["nc"] = build_graph()
    return _GRAPH_CACHE["nc"]


def kernel(nullary_preds, unary_preds, binary_preds, kernel):
    nul = np.asarray(nullary_preds, dtype=np.float32)
    una = np.asarray(unary_preds, dtype=np.float32)
    binf = np.asarray(binary_preds, dtype=np.float32).reshape(B, K, P2)
    ker = np.asarray(kernel, dtype=np.float32)

    nc = get_graph()
    in_maps = []
    for core in range(NCORES):
        bs = slice(core * BL, (core + 1) * BL)
        in_maps.append(make_core_inputs(nul[bs], una[bs], binf[bs], ker))
    res = run_bass_kernel_spmd(nc, in_maps, core_ids=list(range(NCORES)))
    global LAST_RESULTS
    LAST_RESULTS = res
    out = np.concatenate(
        [np.asarray(res.results[i]["out"]).T for i in range(NCORES)], 0
    )
    return out.astype(np.float32)
